# revision 41
# baseline (speedup 1.0000x reference)
"""Multi-head differential attention on 8 trn2 NeuronCores.

Sharding: core c handles batch b = c // 4 and heads [4g, 4g+4) where
g = c % 4 (batch x head-group parallel). All layout transposes and
dtype conversion (f32 -> bf16) happen on the host; each core computes
a partial [S, E] output (its heads' contribution through the output
projection) in bf16, and the host sums the 4 partials per batch in
f32.

Device program (per core, SPMD), all matmul operands bf16 (enables
fast-weight-load so LDWEIGHTS overlaps the matmuls) with f32 PSUM
accumulation:

  V pass: V[s, d4] = x W_v^T computed directly in s-major layout
      (lhsT = xT e-chunk, rhs = wv e-chunk) -- no PE transposes.
  proj pass (per head, per proj q1/q2/k1/k2): qT/kT in [d, S] layout
      (lhsT = weight chunk, rhs = resident xT chunk).
  attention (per head, per 512-wide q block): scores kept transposed
      [k, q]; both streams' score tiles land in one 2-bank PSUM tile
      so a single ACT instruction computes exp of [128, 1024] into a
      bf16 E tile (no max subtraction -- scores are O(5) so exp is
      safe); causal mask applied only on block-diagonal tiles via one
      bf16 DVE multiply; U += V^T E and l += 1^T E accumulate in PSUM;
      normalization uses reciprocal_approx_fast + gpsimd partition
      broadcast, with the differential combine on DVE writing bf16 oT.
  out proj: y[s-tile, :] = sum_h oT_h^T wo_h, streamed out as bf16.
"""

import math
from contextlib import ExitStack

import numpy as np
import ml_dtypes

import concourse.bass as bass
import concourse.tile as tile
from concourse import bacc, mybir
from concourse import bass_utils

B, S, E = 2, 2048, 2048
H, D = 16, 128
HPC = 4                      # heads per core
N_CORES = 8
LAMBDA_INIT = 0.8 - 0.6 * math.exp(-0.3 * H)
SCALE = 1.0 / math.sqrt(D)

F32 = mybir.dt.float32
BF16 = mybir.dt.bfloat16

NE = E // 128                # contraction chunks of 128
NSB = S // 512               # 512-wide s blocks
NST = S // 128               # 128-wide s tiles
NQB = S // 512               # attention q blocks of 512

_cached = {}
TRACE = False


def _build():
    nc = bacc.Bacc(
        "TRN2",
        target_bir_lowering=False,
        debug=False,
        enable_asserts=False,
        num_devices=N_CORES,
    )

    x_d = nc.dram_tensor("x_lay", [128, NE, S], BF16, kind="ExternalInput").ap()
    wqk_d = nc.dram_tensor("wqk", [4, HPC, 128, NE, 128], BF16,
                           kind="ExternalInput").ap()
    wv_d = nc.dram_tensor("wv_lay", [128, NE, HPC * 128], BF16,
                          kind="ExternalInput").ap()
    wo_d = nc.dram_tensor("wo_lay", [HPC, 128, E], BF16,
                          kind="ExternalInput").ap()
    mask_d = nc.dram_tensor("mask1", [128, 128], BF16,
                            kind="ExternalInput").ap()
    ones_d = nc.dram_tensor("ones", [128, 1], BF16, kind="ExternalInput").ap()
    lam_d = nc.dram_tensor("lam", [1, 1], F32, kind="ExternalInput").ap()
    y_d = nc.dram_tensor("y", [S, E], BF16, kind="ExternalOutput").ap()

    with tile.TileContext(nc) as tc, ExitStack() as ctx:
        const = ctx.enter_context(tc.tile_pool(name="const", bufs=1))
        qkp = ctx.enter_context(tc.tile_pool(name="qkp", bufs=1))
        vp = ctx.enter_context(tc.tile_pool(name="vp", bufs=1))
        otp = ctx.enter_context(tc.tile_pool(name="otp", bufs=1))
        # single PSUM pool for the whole kernel: projections/V/outproj cycle
        # through the same u1/u2 (and sca/scb) tags the attention uses, so
        # there is no pool-boundary bank handoff between phases
        ps = ctx.enter_context(tc.tile_pool(name="ps", bufs=1, space="PSUM"))

        # ---- constants
        ones_sb = const.tile([128, 1], BF16, name="ones")
        nc.sync.dma_start(out=ones_sb, in_=ones_d)
        lam_sb = const.tile([1, 1], F32, name="lam")
        nc.sync.dma_start(out=lam_sb, in_=lam_d)

        # ---- persistent SBUF tensors
        # qT1/qT2/kT1/kT2 per head, [128 d, S] bf16
        qk = [[qkp.tile([128, S], BF16, name=f"qk{p}h{h}", tag=f"qk{p}h{h}")
               for p in range(4)] for h in range(HPC)]
        # V in s-major layout: [128 s, st, 4h*128 d] bf16
        V = vp.tile([128, NST, HPC * 128], BF16, name="V")
        # per-head attention output, [128 d, S] bf16
        oT = [otp.tile([128, S], BF16, name=f"oT{h}", tag=f"oT{h}")
              for h in range(HPC)]

        # ---- phase-scoped pools (closed to free SBUF)
        pactx = ExitStack()
        xp = pactx.enter_context(tc.tile_pool(name="xp", bufs=1))
        wvp = pactx.enter_context(tc.tile_pool(name="wvp", bufs=1))
        wp = pactx.enter_context(tc.tile_pool(name="wp", bufs=7))

        # resident x, [128 e-lo, e-hi, S] bf16; the first projection group
        # needs w(h0,p0) and x s-block 0, so issue those first, split small
        # so they spread across DMA queues, then stream the rest s-block-major
        xT = xp.tile([128, NE, S], BF16, name="xT")
        # head 0's four weight tiles and x s-block 0 first, split small so
        # they spread across DMA queues; then the rest of x s-block-major
        w0s = [wp.tile([128, NE, 128], BF16, tag="w", name=f"w0p{p}")
               for p in range(4)]
        for eq in range(4):
            esl = slice(eq * 4, (eq + 1) * 4)
            nc.sync.dma_start(out=w0s[0][:, esl, :], in_=wqk_d[0, 0, :, esl, :])
        for eq in range(4):
            esl = slice(eq * 4, (eq + 1) * 4)
            nc.sync.dma_start(out=xT[:, esl, 0:512], in_=x_d[:, esl, 0:512])
        for p in range(1, 4):
            for eq in range(4):
                esl = slice(eq * 4, (eq + 1) * 4)
                nc.sync.dma_start(out=w0s[p][:, esl, :], in_=wqk_d[p, 0, :, esl, :])
        for sb in range(1, NSB):
            for eq in range(4):
                nc.sync.dma_start(
                    out=xT[:, eq * 4:(eq + 1) * 4, sb * 512:(sb + 1) * 512],
                    in_=x_d[:, eq * 4:(eq + 1) * 4, sb * 512:(sb + 1) * 512])
        wv_sb = wvp.tile([128, NE, HPC * 128], BF16, name="wv")
        nc.sync.dma_start(out=wv_sb, in_=wv_d)

        pcount = 0

        def _qk_proj(h):
            nonlocal pcount
            # all 4 weight tiles for this head resident (bufs=8 rotates),
            # s-block outer so early groups only need the s-blocks of x
            # that have already landed
            if h == 0:
                ws = w0s
            else:
                ws = []
                for p in range(4):
                    w_sb = wp.tile([128, NE, 128], BF16, tag="w", name=f"w{h}p{p}")
                    for eq in range(4):
                        esl = slice(eq * 4, (eq + 1) * 4)
                        nc.sync.dma_start(out=w_sb[:, esl, :],
                                          in_=wqk_d[p, h, :, esl, :])
                    ws.append(w_sb)
            for sb in range(NSB):
                for p in range(4):
                    acc = ps.tile([128, 512], F32,
                                  tag="u1" if pcount % 2 == 0 else "u2")
                    pcount += 1
                    for e in range(NE):
                        nc.tensor.matmul(acc, ws[p][:, e, :],
                                         xT[:, e, sb * 512:(sb + 1) * 512],
                                         start=(e == 0), stop=(e == NE - 1))
                    nc.vector.tensor_copy(qk[h][p][:, sb * 512:(sb + 1) * 512], acc)

        # head 0 projections first (need only one s-block of x to start),
        # then the V pass (needs all of x), then the remaining heads
        _qk_proj(0)
        for st in range(NST):
            pv = ps.tile([128, 512], F32, tag="u1" if pcount % 2 == 0 else "u2")
            pcount += 1
            for e in range(NE):
                nc.tensor.matmul(pv, xT[:, e, st * 128:(st + 1) * 128],
                                 wv_sb[:, e, :], start=(e == 0), stop=(e == NE - 1))
            nc.vector.tensor_copy(V[:, st, :], pv)
        for h in range(1, HPC):
            _qk_proj(h)

        pactx.close()

        # ---- attention phase pools
        aactx = ExitStack()
        ep = aactx.enter_context(tc.tile_pool(name="ep", bufs=6))
        np_ = aactx.enter_context(tc.tile_pool(name="np", bufs=4))
        wop = aactx.enter_context(tc.tile_pool(name="wop", bufs=1))
        yp = aactx.enter_context(tc.tile_pool(name="yp", bufs=2))

        woT = [wop.tile([128, E], BF16, name=f"wo{h}", tag=f"wo{h}")
               for h in range(HPC)]
        for h in range(HPC):
            nc.sync.dma_start(out=woT[h], in_=wo_d[h])
        mask1 = wop.tile([128, 128], BF16, name="mask1", tag="mask1")
        nc.sync.dma_start(out=mask1, in_=mask_d)

        # q-block-outer, head-inner: each qb cycle mixes the four heads'
        # exp-gated attention with the previous qb's exp-independent output
        # projection, so ScalarE latency never gates the PE
        scnt = 0

        def _attn(qb, h):
            nonlocal scnt
            qT1, qT2, kT1, kT2 = qk[h]
            qsl = slice(qb * 512, (qb + 1) * 512)
            U1 = ps.tile([128, 512], F32, tag="u1")
            U2 = ps.tile([128, 512], F32, tag="u2")
            l1 = ps.tile([1, 512], F32, tag="l1")
            l2 = ps.tile([1, 512], F32, tag="l2")
            nkt = 4 * qb + 4
            ecats = {}

            def _consume(kt):
                # U/l accumulation for tile kt (runs one kt behind the
                # score/exp stage so the exp latency is fully hidden)
                qo, n, Ecat = ecats.pop(kt)
                st_ = (kt == 0)
                sp_ = (kt == nkt - 1)
                vsl = V[:, kt, h * 128:(h + 1) * 128]
                nc.tensor.matmul(U1[:, qo:512], vsl, Ecat[:, 0:n],
                                 start=st_, stop=sp_, skip_group_check=True)
                nc.tensor.matmul(U2[:, qo:512], vsl, Ecat[:, 512:512 + n],
                                 start=st_, stop=sp_, skip_group_check=True)
                nc.tensor.matmul(l1[:, qo:512], ones_sb, Ecat[:, 0:n],
                                 start=st_, stop=sp_, skip_group_check=True)
                nc.tensor.matmul(l2[:, qo:512], ones_sb, Ecat[:, 512:512 + n],
                                 start=st_, stop=sp_, skip_group_check=True)

            for kt in range(nkt):
                ksl = slice(kt * 128, (kt + 1) * 128)
                kl = kt - 4 * qb
                # on block-diagonal tiles, skip the fully-masked
                # q columns: only q >= kl*128 contributes
                qo = max(kl, 0) * 128
                n = 512 - qo
                qsl2 = slice(qb * 512 + qo, (qb + 1) * 512)
                scat = ps.tile([128, 1024], F32,
                               tag="sca" if scnt % 2 == 0 else "scb")
                scnt += 1
                nc.tensor.matmul(scat[:, 0:n], kT1[:, ksl], qT1[:, qsl2])
                nc.tensor.matmul(scat[:, 512:512 + n], kT2[:, ksl],
                                 qT2[:, qsl2])
                Ecat = ep.tile([128, 1024], BF16, tag="E")
                if kl >= 1:
                    # shrunk tile: exp only the written columns
                    nc.scalar.activation(Ecat[:, 0:n], scat[:, 0:n],
                                         mybir.ActivationFunctionType.Exp,
                                         scale=SCALE)
                    nc.scalar.activation(Ecat[:, 512:512 + n],
                                         scat[:, 512:512 + n],
                                         mybir.ActivationFunctionType.Exp,
                                         scale=SCALE)
                else:
                    # single exp over both streams' banks
                    nc.scalar.activation(Ecat, scat,
                                         mybir.ActivationFunctionType.Exp,
                                         scale=SCALE)
                if kl >= 0:
                    # mask the leading 128 q columns (the k-tile's own
                    # diagonal 128x128 block) of each stream
                    nc.vector.tensor_mul(Ecat[:, 0:128], Ecat[:, 0:128],
                                         mask1)
                    nc.vector.tensor_mul(Ecat[:, 512:640], Ecat[:, 512:640],
                                         mask1)
                ecats[kt] = (qo, n, Ecat)
                if kt >= 2:
                    _consume(kt - 2)
            _consume(nkt - 2)
            _consume(nkt - 1)
            # copy U out of PSUM promptly to release the banks, then
            # normalize + differential combine off the critical path
            u1c = np_.tile([128, 512], F32, tag="u1c")
            u2c = np_.tile([128, 512], F32, tag="u2c")
            nc.vector.tensor_copy(u1c, U1)
            nc.vector.tensor_copy(u2c, U2)
            r1 = np_.tile([1, 512], F32, tag="r1")
            r2 = np_.tile([1, 512], F32, tag="r2")
            nc.vector.reciprocal_approx_fast(out=r1, in_=l1)
            nc.vector.reciprocal_approx_fast(out=r2, in_=l2)
            r2l = np_.tile([1, 512], F32, tag="r2l")
            nc.vector.tensor_scalar_mul(r2l, r2, lam_sb[0:1, 0:1])
            r1b = np_.tile([128, 512], F32, tag="r1b")
            r2b = np_.tile([128, 512], F32, tag="r2b")
            nc.gpsimd.partition_broadcast(r1b, r1)
            nc.gpsimd.partition_broadcast(r2b, r2l)
            d1 = np_.tile([128, 512], BF16, tag="d1")
            d2 = np_.tile([128, 512], BF16, tag="d2")
            nc.vector.tensor_mul(d1, u1c, r1b)
            nc.vector.tensor_mul(d2, u2c, r2b)
            nc.vector.tensor_sub(oT[h][:, qsl], d1, d2)

        def _outproj(qb):
            # y[st] = sum_h oT_h[:, st]^T @ woT_h for this q block's s-tiles
            nonlocal scnt
            for st in range(qb * 4, qb * 4 + 4):
                ysb = yp.tile([128, E], BF16, tag="ysb")
                ssl = slice(st * 128, (st + 1) * 128)
                for eb in range(4):
                    esl = slice(eb * 512, (eb + 1) * 512)
                    ypp = ps.tile([128, 512], F32,
                                  tag="sca" if scnt % 2 == 0 else "scb")
                    scnt += 1
                    for hh in range(HPC):
                        nc.tensor.matmul(
                            ypp, oT[hh][:, ssl],
                            woT[hh][:, eb * 512:(eb + 1) * 512],
                            start=(hh == 0), stop=(hh == HPC - 1))
                    if eb % 2 == 0:
                        nc.vector.tensor_copy(ysb[:, esl], ypp)
                    else:
                        nc.scalar.copy(ysb[:, esl], ypp)
                    nc.sync.dma_start(out=y_d[ssl, esl], in_=ysb[:, esl])

        # qb cycles in order [1, 0, 2, 3]: the all-diagonal qb0 cycle is
        # DVE/normalize-chain bound, so give it qb1's output projection as
        # PE filler work; each later cycle carries the previous one's
        qb_order = [1, 0, 2, 3]
        for i, qb in enumerate(qb_order):
            for h in range(HPC):
                _attn(qb, h)
            if i >= 1:
                _outproj(qb_order[i - 1])
        _outproj(qb_order[-1])

        aactx.close()

    nc.compile()
    return nc


def _lay_lhsT(w):
    # [E_contract, M] f32 -> [128 p, NE c, M] bf16 with e = c*128 + p
    e, m = w.shape
    return np.ascontiguousarray(
        w.reshape(NE, 128, m).transpose(1, 0, 2).astype(ml_dtypes.bfloat16))


def kernel(**inputs):
    x = np.asarray(inputs["x"], dtype=np.float32)
    wq = np.asarray(inputs["wq"], dtype=np.float32)
    wk = np.asarray(inputs["wk"], dtype=np.float32)
    wv = np.asarray(inputs["wv"], dtype=np.float32)
    wq2 = np.asarray(inputs["wq2"], dtype=np.float32)
    wk2 = np.asarray(inputs["wk2"], dtype=np.float32)
    wo = np.asarray(inputs["wo"], dtype=np.float32)
    lq1 = np.asarray(inputs["lambda_q1"], dtype=np.float64)
    lk1 = np.asarray(inputs["lambda_k1"], dtype=np.float64)
    lq2 = np.asarray(inputs["lambda_q2"], dtype=np.float64)
    lk2 = np.asarray(inputs["lambda_k2"], dtype=np.float64)

    lam = float(np.exp(np.sum(lq1 * lk1)) - np.exp(np.sum(lq2 * lk2)) + LAMBDA_INIT)

    if "nc" not in _cached:
        _cached["nc"] = _build()
    nc = _cached["nc"]

    # 128x128 lower-triangular mask for the diagonal blocks
    j = np.arange(128, dtype=np.int32)[None, :]
    i = np.arange(128, dtype=np.int32)[:, None]
    mask1 = np.ascontiguousarray((j >= i).astype(ml_dtypes.bfloat16))
    lam_arr = np.full((1, 1), lam, dtype=np.float32)
    ones = np.ones((128, 1), dtype=ml_dtypes.bfloat16)

    # per-batch x layout: [128 p, NE c, S] with e = c*128 + p
    x_lays = [_lay_lhsT(x[b].T) for b in range(B)]

    in_maps = []
    for c in range(N_CORES):
        b = c // 4
        g = c % 4
        hs = g * HPC * D
        # wqk[p, h] = lhsT chunk layout of W[o_slice, :]^T ([E, 128] -> [128, NE, 128])
        wqk = np.stack([
            np.stack([_lay_lhsT(w[hs + h * D:hs + (h + 1) * D, :].T)
                      for h in range(HPC)])
            for w in (wq, wq2, wk, wk2)])
        wv_lay = _lay_lhsT(wv[hs:hs + HPC * D, :].T)          # [128, NE, 512]
        wo_lay = np.stack([
            np.ascontiguousarray(
                wo[:, hs + h * D:hs + (h + 1) * D].T.astype(ml_dtypes.bfloat16))
            for h in range(HPC)])                              # [4, 128, E]
        in_maps.append({
            "x_lay": x_lays[b],
            "wqk": wqk,
            "wv_lay": wv_lay,
            "wo_lay": wo_lay,
            "mask1": mask1,
            "ones": ones,
            "lam": lam_arr,
        })

    res = bass_utils.run_bass_kernel_spmd(
        nc, in_maps, core_ids=list(range(N_CORES)), trace=TRACE)
    _cached["last_result"] = res

    y = np.zeros((B, S, E), dtype=np.float32)
    for c in range(N_CORES):
        y[c // 4] += np.asarray(res.results[c]["y"], dtype=np.float32)
    return y
